# revision 6
# baseline (speedup 1.0000x reference)
"""CNF vector-field kernel for Trainium2 (8 NeuronCores, data-parallel).

Math: for each row b of x [B, 1+D]:
    z      = x[b, 1:]                     # [D]
    h      = tanh(z @ W1 + b1)            # [H]
    x_out  = h @ W2 + b2                  # [D]
    trJ    = sum_k (1 - h_k^2) * c_k      where c_k = sum_i W1[i,k] * W2[k,i]
    out[b] = [-trJ, x_out]

so out[b, 0]  = (h^2) . c - sum(c)   and   out[b, 1:] = h @ W2 + b2.
The Jacobian trace collapses to a precomputable H-vector c — no [D, D]
Jacobian is ever materialized.

Per-core dataflow (B/8 = 4096 rows):
  - batch-contiguous DMA load [128, 32, 65] (partition p holds rows 32p..32p+31)
  - cast z to bf16, PE-transpose each 128-row slot to feature-major [64, 128]
  - mm1: a^T = W1^T z^T via row-tiled K=64 matmuls (bf16, fp32 PSUM)
  - tanh on ScalarE, PSUM -> SBUF, bias fused, bf16 out
  - square on VectorE (bf16 2x mode)
  - mm2: per 128-row block, out = [h; h^2]^T stationary (FWL) @ [W2 | c] moving,
    accumulated over 4 K=128 chunks into PSUM [128, 65] (batch-major)
  - bias row added during the PSUM -> SBUF copy (VectorE, broadcast)
  - batch-contiguous DMA store
"""

import numpy as np
import ml_dtypes

import concourse.bass as bass
from concourse import bacc
import concourse.mybir as mybir
import concourse.tile as tile
from concourse import bass_utils

NCORES = 8
B, D, H = 32768, 64, 256
FW = D + 1          # 65 columns (log-density slot + state)
BC = B // NCORES    # 4096 rows per core
P = 128
NSLOT = BC // P     # 32 slots; slot n on partition p is row 32p + n
NGROUP = NSLOT // 4  # groups of 4 slots

F32 = mybir.dt.float32
BF16 = mybir.dt.bfloat16


def _pos_to_slot(pos: int) -> int:
    # mm1 emits slot (4g + 2*jj + parity) at block position (4g + 2*parity + jj):
    # swap the low two bits to map position back to slot.
    g4, r = pos - (pos % 4), pos % 4
    return g4 + ((r & 1) << 1) + ((r >> 1) & 1)


def _build_module():
    nc = bacc.Bacc("TRN2")

    x_t = nc.dram_tensor("x", [BC, FW], F32, kind="ExternalInput")
    w1_t = nc.dram_tensor("w1dup", [P, 2, P], BF16, kind="ExternalInput")
    w2_t = nc.dram_tensor("w2aug", [P, 4, FW], BF16, kind="ExternalInput")
    b1_t = nc.dram_tensor("b1f", [P, 2], F32, kind="ExternalInput")
    br_t = nc.dram_tensor("brow", [P, FW], F32, kind="ExternalInput")
    y_t = nc.dram_tensor("y", [BC, FW], F32, kind="ExternalOutput")

    x_ap = x_t.rearrange("(p n) f -> p n f", p=P)   # [128, 32, 65]
    y_ap = y_t.rearrange("(p n) f -> p n f", p=P)

    with tile.TileContext(nc) as tc:
        with (
            tc.tile_pool(name="const", bufs=1) as cpool,
            tc.tile_pool(name="xin", bufs=4) as xpool,
            tc.tile_pool(name="z16", bufs=2) as zpool,
            tc.tile_pool(name="zt", bufs=1) as ztpool,
            tc.tile_pool(name="h", bufs=1) as hpool,
            tc.tile_pool(name="out", bufs=2) as opool,
            tc.tile_pool(name="pt", bufs=2, space="PSUM") as ptpool,
            tc.tile_pool(name="pa", bufs=2, space="PSUM") as papool,
            tc.tile_pool(name="po", bufs=3, space="PSUM") as popool,
        ):
            # constants
            w1sb = cpool.tile([P, 2, P], BF16)
            nc.sync.dma_start(w1sb[:], w1_t[:])
            w2sb = cpool.tile([P, 4, FW], BF16)
            nc.sync.dma_start(w2sb[:], w2_t[:])
            b1sb = cpool.tile([P, 2], F32)
            nc.sync.dma_start(b1sb[:], b1_t[:])
            brsb = cpool.tile([P, FW], F32)
            nc.sync.dma_start(brsb[:], br_t[:])
            ident = cpool.tile([P, P], BF16)
            from concourse.masks import make_identity
            make_identity(nc, ident[:])

            # whole-shard SBUF residents
            hT = hpool.tile([P, 2, NSLOT, P], BF16)     # tanh output, feature-major
            hsq = hpool.tile([P, 2, NSLOT, P], BF16)    # h^2
            zT = ztpool.tile([P, NSLOT // 2, P], BF16)  # z^T pairs

            # 4 input DMAs of 8 slots each
            xin = []
            for q in range(4):
                xt = xpool.tile([P, 8, FW], F32, tag="xin")
                nc.sync.dma_start(xt[:], x_ap[:, 8 * q : 8 * q + 8, :])
                xin.append(xt)

            for g in range(NGROUP):
                q, qs = g // 2, (g % 2) * 4          # which xin tile, slot offset in it
                # cast z -> bf16 (VectorE 2x, single-src fp32)
                z16 = zpool.tile([P, 4, D], BF16, tag="z16")
                nc.vector.tensor_copy(z16[:], xin[q][:, qs : qs + 4, 1:FW])

                # PE transpose each slot; pack slot pairs into [128, 128] PSUM
                for jj in range(2):
                    ptile = ptpool.tile([P, P], BF16, tag="pt")
                    for par in range(2):
                        s = 2 * jj + par
                        nc.tensor.transpose(
                            ptile[64 * par : 64 * par + 64, :], z16[:, s, :], ident[:]
                        )
                    nc.vector.tensor_copy(zT[:, 2 * g + jj, :], ptile[:])

                # mm1 + tanh: two row-tiles (parity) x two H-halves
                for hh in range(2):
                    for par in range(2):
                        pa = papool.tile([P, 2, P], F32, tag="pa")
                        nc.tensor.matmul(
                            pa[:],
                            w1sb[64 * par : 64 * par + 64, hh, :],
                            zT[64 * par : 64 * par + 64, 2 * g : 2 * g + 2, :],
                            start=True,
                            stop=True,
                        )
                        pos0 = 4 * g + 2 * par
                        nc.scalar.activation(
                            hT[:, hh, pos0 : pos0 + 2, :],
                            pa[:],
                            mybir.ActivationFunctionType.Tanh,
                            bias=b1sb[:, hh : hh + 1],
                        )
                # square (VectorE bf16 2x) for this group's 4 block positions
                nc.vector.tensor_tensor(
                    hsq[:, :, 4 * g : 4 * g + 4, :],
                    hT[:, :, 4 * g : 4 * g + 4, :],
                    hT[:, :, 4 * g : 4 * g + 4, :],
                    mybir.AluOpType.mult,
                )

                # mm2: per 128-row block position -> batch-major [128, 65]
                ot = opool.tile([P, 4, FW], F32, tag="out")
                for r in range(4):
                    pos = 4 * g + r
                    po = popool.tile([P, FW], F32, tag="po")
                    for k, src in enumerate((hT, hT, hsq, hsq)):
                        nc.tensor.matmul(
                            po[:],
                            src[:, k % 2, pos, :],
                            w2sb[:, k, :],
                            start=(k == 0),
                            stop=(k == 3),
                        )
                    n = _pos_to_slot(pos)
                    nc.vector.tensor_tensor(
                        ot[:, n % 4, :], po[:], brsb[:], mybir.AluOpType.add
                    )
                # store this group's 4 slots (positions map onto the same 4-slot range)
                nc.sync.dma_start(y_ap[:, 4 * g : 4 * g + 4, :], ot[:])

    nc.compile()
    return nc


_NC_CACHE = {}


def _get_module():
    if "nc" not in _NC_CACHE:
        _NC_CACHE["nc"] = _build_module()
    return _NC_CACHE["nc"]


def _host_params(W1, b1, W2, b2):
    c = np.sum(W1.T.astype(np.float64) * W2.astype(np.float64), axis=1)  # [H]

    w1dup = np.zeros((P, 2, P), dtype=ml_dtypes.bfloat16)
    for hh in range(2):
        blk = W1[:, hh * P : (hh + 1) * P].astype(ml_dtypes.bfloat16)
        w1dup[0:64, hh, :] = blk
        w1dup[64:128, hh, :] = blk

    c16 = c.astype(ml_dtypes.bfloat16)
    w2aug = np.zeros((P, 4, FW), dtype=ml_dtypes.bfloat16)
    for k in range(2):
        w2aug[:, k, 1:FW] = W2[k * P : (k + 1) * P, :].astype(ml_dtypes.bfloat16)
    for k in range(2, 4):
        w2aug[:, k, 0] = c16[(k - 2) * P : (k - 1) * P]

    b1f = np.stack([b1[0:P], b1[P : 2 * P]], axis=1).astype(np.float32)  # [128, 2]

    brow = np.zeros((1, FW), dtype=np.float32)
    brow[0, 0] = -np.sum(c16.astype(np.float64)).astype(np.float32)
    brow[0, 1:] = b2.astype(np.float32)
    brow = np.repeat(brow, P, axis=0)  # replicate across partitions
    return w1dup, w2aug, b1f, brow


def kernel(x, W1, b1, W2, b2):
    x = np.asarray(x, dtype=np.float32)
    W1 = np.asarray(W1, dtype=np.float32)
    b1 = np.asarray(b1, dtype=np.float32)
    W2 = np.asarray(W2, dtype=np.float32)
    b2 = np.asarray(b2, dtype=np.float32)

    w1dup, w2aug, b1f, brow = _host_params(W1, b1, W2, b2)
    nc = _get_module()

    in_maps = []
    for core in range(NCORES):
        shard = np.ascontiguousarray(x[core * BC : (core + 1) * BC, :])
        in_maps.append(
            {"x": shard, "w1dup": w1dup, "w2aug": w2aug, "b1f": b1f, "brow": brow}
        )

    res = bass_utils.run_bass_kernel_spmd(nc, in_maps, core_ids=list(range(NCORES)))
    _NC_CACHE["last_results"] = res
    out = np.concatenate([r["y"] for r in res.results], axis=0)
    return out.astype(np.float32)


if __name__ == "__main__":
    rng = np.random.default_rng(0)
    x = rng.standard_normal((B, FW), dtype=np.float32)
    W1 = (rng.standard_normal((D, H)) / np.sqrt(D)).astype(np.float32)
    b1 = np.zeros(H, np.float32)
    W2 = (rng.standard_normal((H, D)) / np.sqrt(H)).astype(np.float32)
    b2 = np.zeros(D, np.float32)
    y = kernel(x=x, W1=W1, b1=b1, W2=W2, b2=b2)
    print(y.shape, y.dtype)


# revision 27
# speedup vs baseline: 1.3610x; 1.3610x over previous
"""CNF vector-field kernel for Trainium2 (8 NeuronCores, data-parallel).

Math: for each row b of x [B, 1+D]:
    z      = x[b, 1:]                     # [D]
    h      = tanh(z @ W1 + b1)            # [H]
    x_out  = h @ W2 + b2                  # [D]
    trJ    = sum_k (1 - h_k^2) * c_k      where c_k = sum_i W1[i,k] * W2[k,i]
    out[b] = [-trJ, x_out]

so out[b, 0]  = (h^2) . c - sum(c)   and   out[b, 1:] = h @ W2 + b2.
The Jacobian trace collapses to a precomputable H-vector c — no [D, D]
Jacobian is ever materialized.

Per-core dataflow (B/8 = 4096 rows = 32 slots of 128; slot n on
partition p is row 32p + n):
  - SWDGE cast-DMA load: full x rows fp32 -> bf16, each 65-wide slot padded
    into a 128-wide window of SBUF
  - XBAR DMA-transpose each [128, 128] window -> z^T (row 0 = the log-density
    column; killed in mm1 by a zero row in the padded W1)
  - mm1: a^T = W1pad^T zT, K=65, N=512 (bf16, fp32 PSUM spanning 2 banks)
  - tanh on ScalarE: [128, 1024] PSUM -> SBUF bf16, b1 bias fused
  - square on VectorE (bf16 2x)
  - mm2 batch-major: per 128-row slot, lhsT = h/hsq block (stationary, FWL),
    rhs = [W2 | c] chunks, 4 x K=128 accumulated into a quarter PSUM bank
  - bias row + PSUM->SBUF copy on VectorE, 4 slots per instruction
  - contiguous DMA store
"""

import numpy as np
import ml_dtypes

import concourse.bass as bass
from concourse import bacc
import concourse.mybir as mybir
import concourse.tile as tile
from concourse import bass_utils
from concourse.tile_rust import add_dep_helper

NCORES = 8
B, D, H = 32768, 64, 256
FW = D + 1          # 65
BC = B // NCORES    # 4096
P = 128
NSLOT = BC // P     # 32
NSG = 4             # supergroups of 8 slots

F32 = mybir.dt.float32
BF16 = mybir.dt.bfloat16


def _build_module():
    nc = bacc.Bacc("TRN2", enable_partition_id=False, enable_asserts=False)

    x_t = nc.dram_tensor("x", [BC, FW], F32, kind="ExternalInput")
    w1_t = nc.dram_tensor("w1pad", [P, 2, P], BF16, kind="ExternalInput")
    w2_t = nc.dram_tensor("w2aug", [P, 4, FW], BF16, kind="ExternalInput")
    b1_t = nc.dram_tensor("b1f", [P, 2], F32, kind="ExternalInput")
    br_t = nc.dram_tensor("brow4", [P, 4, FW], F32, kind="ExternalInput")
    y_t = nc.dram_tensor("y", [BC, FW], F32, kind="ExternalOutput")

    x_ap = x_t.rearrange("(p n) f -> p n f", p=P)   # [128, 32, 65]
    y_ap = y_t.rearrange("(p n) f -> p n f", p=P)

    with tile.TileContext(nc) as tc:
        with (
            tc.tile_pool(name="const", bufs=1) as cpool,
            tc.tile_pool(name="xin", bufs=4) as xpool,
            tc.tile_pool(name="zt", bufs=1) as ztpool,
            tc.tile_pool(name="h", bufs=1) as hpool,
            tc.tile_pool(name="out", bufs=2) as opool,
            tc.tile_pool(name="pa", bufs=3, space="PSUM") as papool,
            tc.tile_pool(name="pt", bufs=2, space="PSUM") as ptpool,
            tc.tile_pool(name="po", bufs=3, space="PSUM") as popool,
        ):
            # input loads first (Sync queue), consts on Scalar queue
            xfs, dmas = [], []
            for q in range(4):
                xf = xpool.tile([P, 8, FW], F32, tag=f"xf{q}")
                dmas.append(nc.sync.dma_start(xf[:], x_ap[:, 8 * q : 8 * q + 8, :]))
                xfs.append(xf)
            w1sb = cpool.tile([P, 2, P], BF16)
            d_w1 = nc.scalar.dma_start(w1sb[:], w1_t[:])
            b1sb = cpool.tile([P, 2], F32)
            d_b1 = nc.scalar.dma_start(b1sb[:], b1_t[:])
            w2sb = cpool.tile([P, 4, FW], BF16)
            d_w2 = nc.scalar.dma_start(w2sb[:], w2_t[:])
            brsb = cpool.tile([P, 4, FW], F32)
            d_br = nc.scalar.dma_start(brsb[:], br_t[:])

            # PE warm-up: ~4.5us of dummy matmuls so HAM reaches 2.4 GHz
            # before the real pipeline arrives
            ident = cpool.tile([P, P], BF16)
            from concourse.masks import make_identity
            make_identity(nc, ident[:])

            warm = cpool.tile([P, 512], BF16)
            nc.gpsimd.memset(warm[:], 0.0)
            for _ in range(6):
                pwarm = papool.tile([P, 512], F32, tag="pa")
                nc.tensor.matmul(
                    pwarm[:], warm[:, 0:P], warm[:], start=True, stop=True
                )

            # whole-shard SBUF residents
            hT = hpool.tile([P, 2, NSLOT, P], BF16)     # tanh out, feature-major
            hsq = hpool.tile([P, 2, NSLOT, P], BF16)    # h^2
            zT = ztpool.tile([P, NSLOT, P], BF16)       # z^T per slot (row0=junk col)

            # cast-pad each 65-wide slot into a 128-wide bf16 window (DVE),
            # then two batched XBAR transposes of 16 windows each
            xpad = ztpool.tile([P, NSLOT, P], BF16)
            # slots 0-15: PE transposes (low latency; keeps PE warm)
            for half in range(4):
                nc.vector.tensor_copy(
                    xpad[:, 4 * half : 4 * half + 4, 0:FW],
                    xfs[half // 2][:, 4 * (half % 2) : 4 * (half % 2) + 4, :],
                )
                pt = ptpool.tile([P, 4, P], BF16, tag="pt")
                for j in range(4):
                    n = 4 * half + j
                    nc.tensor.transpose(pt[:, j, :], xpad[:, n, :], ident[:])
                nc.vector.tensor_copy(zT[:, 4 * half : 4 * half + 4, :], pt[:])
            # slots 16-31: two batched XBAR transposes, overlapped with compute
            nc.vector.tensor_copy(xpad[:, 16:24, 0:FW], xfs[2][:])
            t0 = nc.sync.dma_start_transpose(
                zT[:, 16:24, :],
                xpad[:, 16:24, :].rearrange("p n f -> p (n f)"),
            )
            # keep every plain DMA before the first transpose: one xbar-mode
            # transition instead of scheduler-chosen interleavings that stall
            for d in dmas + [d_w1, d_b1, d_w2, d_br]:
                add_dep_helper(t0.ins, d.ins, reason="group copies before xbar")
            nc.vector.tensor_copy(xpad[:, 24:NSLOT, 0:FW], xfs[3][:])
            nc.sync.dma_start_transpose(
                zT[:, 24:NSLOT, :],
                xpad[:, 24:NSLOT, :].rearrange("p n f -> p (n f)"),
            )

            for g in range(8):
                # mm1 (K=65: row 0 of W1pad is zero, kills the col-0 junk row)
                for hh in range(2):
                    pa = papool.tile([P, 512], F32, tag="pa")
                    nc.tensor.matmul(
                        pa[:],
                        w1sb[0:FW, hh, :],
                        zT[0:FW, 4 * g : 4 * g + 4, :],
                        start=True,
                        stop=True,
                    )
                    nc.scalar.activation(
                        hT[:, hh, 4 * g : 4 * g + 4, :],
                        pa[:],
                        mybir.ActivationFunctionType.Tanh,
                        bias=b1sb[:, hh : hh + 1],
                    )

                # square (bf16 2x)
                nc.vector.tensor_tensor(
                    hsq[:, :, 4 * g : 4 * g + 4, :],
                    hT[:, :, 4 * g : 4 * g + 4, :],
                    hT[:, :, 4 * g : 4 * g + 4, :],
                    mybir.AluOpType.mult,
                )

                # mm2: batch-major, 4 slots per PSUM bank
                if g % 2 == 0:
                    ot = opool.tile([P, 8, FW], F32, tag="out")
                po = popool.tile([P, 4, FW], F32, tag="po")
                for j in range(4):
                    n = 4 * g + j
                    for k, src in enumerate((hT, hT, hsq, hsq)):
                        nc.tensor.matmul(
                            po[:, j, :],
                            src[:, k % 2, n, :],
                            w2sb[:, k, :],
                            start=(k == 0),
                            stop=(k == 3),
                        )
                nc.vector.tensor_tensor(
                    ot[:, 4 * (g % 2) : 4 * (g % 2) + 4, :],
                    po[:],
                    brsb[:],
                    mybir.AluOpType.add,
                )
                if g >= 6:
                    nc.sync.dma_start(
                        y_ap[:, 4 * g : 4 * g + 4, :], ot[:, 4 * (g % 2) : 4 * (g % 2) + 4, :]
                    )
                elif g % 2 == 1:
                    nc.sync.dma_start(
                        y_ap[:, 8 * (g // 2) : 8 * (g // 2) + 8, :], ot[:]
                    )

    nc.compile()
    return nc


_NC_CACHE = {}


def _get_module():
    if "nc" not in _NC_CACHE:
        _NC_CACHE["nc"] = _build_module()
    return _NC_CACHE["nc"]


def _host_params(W1, b1, W2, b2):
    c = np.sum(W1.T.astype(np.float64) * W2.astype(np.float64), axis=1)  # [H]

    w1pad = np.zeros((P, 2, P), dtype=ml_dtypes.bfloat16)
    for hh in range(2):
        w1pad[1 : D + 1, hh, :] = W1[:, hh * P : (hh + 1) * P].astype(
            ml_dtypes.bfloat16
        )

    c16 = c.astype(ml_dtypes.bfloat16)
    w2aug = np.zeros((P, 4, FW), dtype=ml_dtypes.bfloat16)
    for k in range(2):
        w2aug[:, k, 1:FW] = W2[k * P : (k + 1) * P, :].astype(ml_dtypes.bfloat16)
    for k in range(2, 4):
        w2aug[:, k, 0] = c16[(k - 2) * P : (k - 1) * P]

    b1f = np.stack([b1[0:P], b1[P : 2 * P]], axis=1).astype(np.float32)  # [128, 2]

    brow = np.zeros((1, 1, FW), dtype=np.float32)
    brow[0, 0, 0] = -np.sum(c16.astype(np.float64)).astype(np.float32)
    brow[0, 0, 1:] = b2.astype(np.float32)
    brow4 = np.broadcast_to(brow, (P, 4, FW)).copy()
    return w1pad, w2aug, b1f, brow4


def kernel(x, W1, b1, W2, b2):
    x = np.asarray(x, dtype=np.float32)
    W1 = np.asarray(W1, dtype=np.float32)
    b1 = np.asarray(b1, dtype=np.float32)
    W2 = np.asarray(W2, dtype=np.float32)
    b2 = np.asarray(b2, dtype=np.float32)

    w1pad, w2aug, b1f, brow4 = _host_params(W1, b1, W2, b2)
    nc = _get_module()

    in_maps = []
    for core in range(NCORES):
        shard = np.ascontiguousarray(x[core * BC : (core + 1) * BC, :])
        in_maps.append(
            {"x": shard, "w1pad": w1pad, "w2aug": w2aug, "b1f": b1f, "brow4": brow4}
        )

    res = bass_utils.run_bass_kernel_spmd(nc, in_maps, core_ids=list(range(NCORES)))
    _NC_CACHE["last_results"] = res
    out = np.concatenate([r["y"] for r in res.results], axis=0)
    return out.astype(np.float32)


if __name__ == "__main__":
    rng = np.random.default_rng(0)
    x = rng.standard_normal((B, FW), dtype=np.float32)
    W1 = (rng.standard_normal((D, H)) / np.sqrt(D)).astype(np.float32)
    b1 = np.zeros(H, np.float32)
    W2 = (rng.standard_normal((H, D)) / np.sqrt(H)).astype(np.float32)
    b2 = np.zeros(D, np.float32)
    y = kernel(x=x, W1=W1, b1=b1, W2=W2, b2=b2)
    print(y.shape, y.dtype)


# revision 28
# speedup vs baseline: 1.4307x; 1.0512x over previous
"""CNF vector-field kernel for Trainium2 (8 NeuronCores, data-parallel).

Math: for each row b of x [B, 1+D]:
    z      = x[b, 1:]                     # [D]
    h      = tanh(z @ W1 + b1)            # [H]
    x_out  = h @ W2 + b2                  # [D]
    trJ    = sum_k (1 - h_k^2) * c_k      where c_k = sum_i W1[i,k] * W2[k,i]
    out[b] = [-trJ, x_out]

so out[b, 0]  = (h^2) . c - sum(c)   and   out[b, 1:] = h @ W2 + b2.
The Jacobian trace collapses to a precomputable H-vector c — no [D, D]
Jacobian is ever materialized.

Per-core dataflow (B/8 = 4096 rows = 32 slots of 128; slot n on
partition p is row 32p + n):
  - SWDGE cast-DMA load: full x rows fp32 -> bf16, each 65-wide slot padded
    into a 128-wide window of SBUF
  - XBAR DMA-transpose each [128, 128] window -> z^T (row 0 = the log-density
    column; killed in mm1 by a zero row in the padded W1)
  - mm1: a^T = W1pad^T zT, K=65, N=512 (bf16, fp32 PSUM spanning 2 banks)
  - tanh on ScalarE: [128, 1024] PSUM -> SBUF bf16, b1 bias fused
  - square on VectorE (bf16 2x)
  - mm2 batch-major: per 128-row slot, lhsT = h/hsq block (stationary, FWL),
    rhs = [W2 | c] chunks, 4 x K=128 accumulated into a quarter PSUM bank
  - bias row + PSUM->SBUF copy on VectorE, 4 slots per instruction
  - contiguous DMA store
"""

import numpy as np
import ml_dtypes

import concourse.bass as bass
from concourse import bacc
import concourse.mybir as mybir
import concourse.tile as tile
from concourse import bass_utils
from concourse.tile_rust import add_dep_helper

NCORES = 8
B, D, H = 32768, 64, 256
FW = D + 1          # 65
BC = B // NCORES    # 4096
P = 128
NSLOT = BC // P     # 32
NSG = 4             # supergroups of 8 slots

F32 = mybir.dt.float32
BF16 = mybir.dt.bfloat16


def _build_module():
    nc = bacc.Bacc("TRN2")

    x_t = nc.dram_tensor("x", [BC, FW], F32, kind="ExternalInput")
    w1_t = nc.dram_tensor("w1pad", [P, 2, P], BF16, kind="ExternalInput")
    w2_t = nc.dram_tensor("w2aug", [P, 4, FW], BF16, kind="ExternalInput")
    b1_t = nc.dram_tensor("b1f", [P, 2], F32, kind="ExternalInput")
    br_t = nc.dram_tensor("brow4", [P, 4, FW], F32, kind="ExternalInput")
    y_t = nc.dram_tensor("y", [BC, FW], F32, kind="ExternalOutput")

    x_ap = x_t.rearrange("(p n) f -> p n f", p=P)   # [128, 32, 65]
    y_ap = y_t.rearrange("(p n) f -> p n f", p=P)

    with tile.TileContext(nc) as tc:
        with (
            tc.tile_pool(name="const", bufs=1) as cpool,
            tc.tile_pool(name="xin", bufs=4) as xpool,
            tc.tile_pool(name="zt", bufs=1) as ztpool,
            tc.tile_pool(name="h", bufs=1) as hpool,
            tc.tile_pool(name="out", bufs=2) as opool,
            tc.tile_pool(name="pa", bufs=3, space="PSUM") as papool,
            tc.tile_pool(name="pt", bufs=2, space="PSUM") as ptpool,
            tc.tile_pool(name="po", bufs=3, space="PSUM") as popool,
        ):
            # input loads first (Sync queue), consts on Scalar queue
            xfs, dmas = [], []
            for q in range(4):
                xf = xpool.tile([P, 8, FW], F32, tag=f"xf{q}")
                dmas.append(nc.sync.dma_start(xf[:], x_ap[:, 8 * q : 8 * q + 8, :]))
                xfs.append(xf)
            w1sb = cpool.tile([P, 2, P], BF16)
            d_w1 = nc.scalar.dma_start(w1sb[:], w1_t[:])
            b1sb = cpool.tile([P, 2], F32)
            d_b1 = nc.scalar.dma_start(b1sb[:], b1_t[:])
            w2sb = cpool.tile([P, 4, FW], BF16)
            d_w2 = nc.scalar.dma_start(w2sb[:], w2_t[:])
            brsb = cpool.tile([P, 4, FW], F32)
            d_br = nc.scalar.dma_start(brsb[:], br_t[:])

            # PE warm-up: ~4.5us of dummy matmuls so HAM reaches 2.4 GHz
            # before the real pipeline arrives
            ident = cpool.tile([P, P], BF16)
            from concourse.masks import make_identity
            make_identity(nc, ident[:])

            warm = cpool.tile([P, 512], BF16)
            nc.gpsimd.memset(warm[:], 0.0)
            for _ in range(6):
                pwarm = papool.tile([P, 512], F32, tag="pa")
                nc.tensor.matmul(
                    pwarm[:], warm[:, 0:P], warm[:], start=True, stop=True
                )

            # whole-shard SBUF residents
            hT = hpool.tile([P, 2, NSLOT, P], BF16)     # tanh out, feature-major
            hsq = hpool.tile([P, 2, NSLOT, P], BF16)    # h^2
            zT = ztpool.tile([P, NSLOT, P], BF16)       # z^T per slot (row0=junk col)

            # cast-pad each 65-wide slot into a 128-wide bf16 window (DVE),
            # then two batched XBAR transposes of 16 windows each
            xpad = ztpool.tile([P, NSLOT, P], BF16)
            # slots 0-15: PE transposes (low latency; keeps PE warm)
            for half in range(4):
                nc.vector.tensor_copy(
                    xpad[:, 4 * half : 4 * half + 4, 0:FW],
                    xfs[half // 2][:, 4 * (half % 2) : 4 * (half % 2) + 4, :],
                )
                pt = ptpool.tile([P, 4, P], BF16, tag="pt")
                for j in range(4):
                    n = 4 * half + j
                    nc.tensor.transpose(pt[:, j, :], xpad[:, n, :], ident[:])
                nc.vector.tensor_copy(zT[:, 4 * half : 4 * half + 4, :], pt[:])
            # slots 16-31: two batched XBAR transposes, overlapped with compute
            nc.vector.tensor_copy(xpad[:, 16:24, 0:FW], xfs[2][:])
            t0 = nc.sync.dma_start_transpose(
                zT[:, 16:24, :],
                xpad[:, 16:24, :].rearrange("p n f -> p (n f)"),
            )
            # keep every plain DMA before the first transpose: one xbar-mode
            # transition instead of scheduler-chosen interleavings that stall
            for d in dmas + [d_w1, d_b1, d_w2, d_br]:
                add_dep_helper(t0.ins, d.ins, reason="group copies before xbar")
            nc.vector.tensor_copy(xpad[:, 24:NSLOT, 0:FW], xfs[3][:])
            nc.sync.dma_start_transpose(
                zT[:, 24:NSLOT, :],
                xpad[:, 24:NSLOT, :].rearrange("p n f -> p (n f)"),
            )

            for g in range(8):
                # mm1 (K=65: row 0 of W1pad is zero, kills the col-0 junk row)
                for hh in range(2):
                    pa = papool.tile([P, 512], F32, tag="pa")
                    nc.tensor.matmul(
                        pa[:],
                        w1sb[0:FW, hh, :],
                        zT[0:FW, 4 * g : 4 * g + 4, :],
                        start=True,
                        stop=True,
                    )
                    nc.scalar.activation(
                        hT[:, hh, 4 * g : 4 * g + 4, :],
                        pa[:],
                        mybir.ActivationFunctionType.Tanh,
                        bias=b1sb[:, hh : hh + 1],
                    )

                # square (bf16 2x)
                nc.vector.tensor_tensor(
                    hsq[:, :, 4 * g : 4 * g + 4, :],
                    hT[:, :, 4 * g : 4 * g + 4, :],
                    hT[:, :, 4 * g : 4 * g + 4, :],
                    mybir.AluOpType.mult,
                )

                # mm2: batch-major, 4 slots per PSUM bank
                if g % 2 == 0:
                    ot = opool.tile([P, 8, FW], F32, tag="out")
                po = popool.tile([P, 4, FW], F32, tag="po")
                for j in range(4):
                    n = 4 * g + j
                    for k, src in enumerate((hT, hT, hsq, hsq)):
                        nc.tensor.matmul(
                            po[:, j, :],
                            src[:, k % 2, n, :],
                            w2sb[:, k, :],
                            start=(k == 0),
                            stop=(k == 3),
                        )
                nc.vector.tensor_tensor(
                    ot[:, 4 * (g % 2) : 4 * (g % 2) + 4, :],
                    po[:],
                    brsb[:],
                    mybir.AluOpType.add,
                )
                if g >= 6:
                    nc.sync.dma_start(
                        y_ap[:, 4 * g : 4 * g + 4, :], ot[:, 4 * (g % 2) : 4 * (g % 2) + 4, :]
                    )
                elif g % 2 == 1:
                    nc.sync.dma_start(
                        y_ap[:, 8 * (g // 2) : 8 * (g // 2) + 8, :], ot[:]
                    )

    nc.compile()
    return nc


_NC_CACHE = {}


def _get_module():
    if "nc" not in _NC_CACHE:
        _NC_CACHE["nc"] = _build_module()
    return _NC_CACHE["nc"]


def _host_params(W1, b1, W2, b2):
    c = np.sum(W1.T.astype(np.float64) * W2.astype(np.float64), axis=1)  # [H]

    w1pad = np.zeros((P, 2, P), dtype=ml_dtypes.bfloat16)
    for hh in range(2):
        w1pad[1 : D + 1, hh, :] = W1[:, hh * P : (hh + 1) * P].astype(
            ml_dtypes.bfloat16
        )

    c16 = c.astype(ml_dtypes.bfloat16)
    w2aug = np.zeros((P, 4, FW), dtype=ml_dtypes.bfloat16)
    for k in range(2):
        w2aug[:, k, 1:FW] = W2[k * P : (k + 1) * P, :].astype(ml_dtypes.bfloat16)
    for k in range(2, 4):
        w2aug[:, k, 0] = c16[(k - 2) * P : (k - 1) * P]

    b1f = np.stack([b1[0:P], b1[P : 2 * P]], axis=1).astype(np.float32)  # [128, 2]

    brow = np.zeros((1, 1, FW), dtype=np.float32)
    brow[0, 0, 0] = -np.sum(c16.astype(np.float64)).astype(np.float32)
    brow[0, 0, 1:] = b2.astype(np.float32)
    brow4 = np.broadcast_to(brow, (P, 4, FW)).copy()
    return w1pad, w2aug, b1f, brow4


def kernel(x, W1, b1, W2, b2):
    x = np.asarray(x, dtype=np.float32)
    W1 = np.asarray(W1, dtype=np.float32)
    b1 = np.asarray(b1, dtype=np.float32)
    W2 = np.asarray(W2, dtype=np.float32)
    b2 = np.asarray(b2, dtype=np.float32)

    w1pad, w2aug, b1f, brow4 = _host_params(W1, b1, W2, b2)
    nc = _get_module()

    in_maps = []
    for core in range(NCORES):
        shard = np.ascontiguousarray(x[core * BC : (core + 1) * BC, :])
        in_maps.append(
            {"x": shard, "w1pad": w1pad, "w2aug": w2aug, "b1f": b1f, "brow4": brow4}
        )

    res = bass_utils.run_bass_kernel_spmd(nc, in_maps, core_ids=list(range(NCORES)))
    _NC_CACHE["last_results"] = res
    out = np.concatenate([r["y"] for r in res.results], axis=0)
    return out.astype(np.float32)


if __name__ == "__main__":
    rng = np.random.default_rng(0)
    x = rng.standard_normal((B, FW), dtype=np.float32)
    W1 = (rng.standard_normal((D, H)) / np.sqrt(D)).astype(np.float32)
    b1 = np.zeros(H, np.float32)
    W2 = (rng.standard_normal((H, D)) / np.sqrt(H)).astype(np.float32)
    b2 = np.zeros(D, np.float32)
    y = kernel(x=x, W1=W1, b1=b1, W2=W2, b2=b2)
    print(y.shape, y.dtype)
